# revision 5
# baseline (speedup 1.0000x reference)
"""Trainium2 Bass kernel for nn_Attention_52604759441672.

Dense causal self-attention block (LayerNorm -> QKV -> RoPE -> causal
softmax attention -> output projection) for x of shape (2, 2048, 1024),
16 heads x 64 dim. Sharded over 8 NeuronCores: data parallel over the
2 batches x tensor parallel over 4 head-groups (4 heads each). Each core
computes its batch's LayerNorm, its head-group's QKV projections,
attention, and a partial output projection; the host sums the 4 partial
outputs per batch.

Internal layouts (per core):
  xn^T   [128, 8, 2048]   d-major (transposed) normalized activations, fp32r
  ropeq  [128, 2, 2048]   2 head-pairs stacked (64 dims each), fp32r
  v_aug  [128, 16, 4, 65] token-major V per k-tile/head + ones column
                          (ones row gives the softmax denominator for free)
  scores are computed transposed: S^T[k, q] so softmax's sum runs through
  the PE via the ones column; exp runs on the scalar engine straight out
  of PSUM.
"""

import os
import sys

for _p in ("/opt/trn_rl_repo",):
    if _p not in sys.path and os.path.isdir(_p):
        sys.path.insert(0, _p)

import numpy as np

import concourse.bass as bass
import concourse.mybir as mybir
import concourse.tile as tile
from concourse import bacc, bass_utils

F32 = mybir.dt.float32
F32R = mybir.dt.float32r
AF = mybir.ActivationFunctionType
ALU = mybir.AluOpType

N_CORES = 8
N = 2048          # sequence length
DIM = 1024        # model dim
DH = 64           # head dim
HPC = 4           # heads per core
HG = HPC * DH     # head-group width = 256
NT = N // 128     # 16 token tiles
KC = DIM // 128   # 8 contraction chunks
CH = N // 512     # 4 q-chunks of 512
SCALE = DH ** -0.5

_CACHE = {}


def _rope_tables():
    inv_freq = 1.0 / (10000.0 ** (np.arange(0, DH, 2, dtype=np.float64) / DH))
    freqs = np.arange(N, dtype=np.float64)[:, None] * inv_freq[None, :]  # (N, 32)
    cos32 = np.cos(freqs).astype(np.float32).T     # (32, N)
    sin32 = np.sin(freqs).astype(np.float32).T     # (32, N)
    cos64 = np.concatenate([cos32, cos32], axis=0)             # (64, N)
    sin64s = np.concatenate([-sin32, sin32], axis=0)           # signed
    cos128 = np.ascontiguousarray(np.tile(cos64, (2, 1)))      # (128, N)
    sin128 = np.ascontiguousarray(np.tile(sin64s, (2, 1)))
    return cos128, sin128


def build_nc():
    nc = bacc.Bacc("TRN2", target_bir_lowering=False, debug=False,
                   enable_asserts=True, num_devices=N_CORES)
    x_d = nc.dram_tensor("x", [N, DIM], F32, kind="ExternalInput").ap()
    wq_d = nc.dram_tensor("wq", [DIM, HG], F32, kind="ExternalInput").ap()
    wk_d = nc.dram_tensor("wk", [DIM, HG], F32, kind="ExternalInput").ap()
    wv_d = nc.dram_tensor("wv", [DIM, HG], F32, kind="ExternalInput").ap()
    wo_d = nc.dram_tensor("wo", [HG, DIM], F32, kind="ExternalInput").ap()
    cos_d = nc.dram_tensor("cos", [128, N], F32, kind="ExternalInput").ap()
    sin_d = nc.dram_tensor("sin", [128, N], F32, kind="ExternalInput").ap()
    tri_d = nc.dram_tensor("tri", [128, 128], F32, kind="ExternalInput").ap()
    onez_d = nc.dram_tensor("onez", [128, 384], F32, kind="ExternalInput").ap()
    ident_d = nc.dram_tensor("ident", [128, 128], F32, kind="ExternalInput").ap()
    out_d = nc.dram_tensor("out", [N, DIM], F32, kind="ExternalOutput").ap()

    with tile.TileContext(nc) as tc:
        _emit(nc, tc, x_d, wq_d, wk_d, wv_d, wo_d, cos_d, sin_d, tri_d,
              ident_d, out_d, onez_d)
    nc.compile()
    return nc


def _emit(nc, tc, x_d, wq_d, wk_d, wv_d, wo_d, cos_d, sin_d, tri_d, ident_d,
          out_d, onez_d):
    from contextlib import ExitStack
    ctx = ExitStack()
    with ctx:
        consts = ctx.enter_context(tc.tile_pool(name="consts", bufs=1))
        wpool = ctx.enter_context(tc.tile_pool(name="wpool", bufs=1))
        xnpool = ctx.enter_context(tc.tile_pool(name="xnpool", bufs=1))
        persist = ctx.enter_context(tc.tile_pool(name="persist", bufs=1))

        # ---- constants ----
        wo_sb = consts.tile([128, 2, DIM], F32R)
        nc.sync.dma_start(out=wo_sb, in_=wo_d.bitcast(F32R).rearrange(
            "(c p) f -> p c f", p=128))
        tri_sb = consts.tile([128, 128], F32)
        nc.sync.dma_start(out=tri_sb, in_=tri_d)
        ident_sb = consts.tile([128, 128], F32R)
        nc.sync.dma_start(out=ident_sb, in_=ident_d.bitcast(F32R))
        eps_sb = consts.tile([128, 1], F32)
        nc.vector.memset(eps_sb, 1e-5)
        zer_sb = consts.tile([128, 384], F32R)
        nc.vector.memset(zer_sb.bitcast(F32), 0.0)

        cos_sb = wpool.tile([128, N], F32)
        nc.sync.dma_start(out=cos_sb, in_=cos_d)
        sin_sb = wpool.tile([128, N], F32)
        nc.sync.dma_start(out=sin_sb, in_=sin_d)
        wq_sb = wpool.tile([128, KC, HG], F32R)
        nc.sync.dma_start(out=wq_sb, in_=wq_d.bitcast(F32R).rearrange(
            "(kc p) f -> p kc f", p=128))
        wk_sb = wpool.tile([128, KC, HG], F32R)
        nc.sync.dma_start(out=wk_sb, in_=wk_d.bitcast(F32R).rearrange(
            "(kc p) f -> p kc f", p=128))
        wv_sb = wpool.tile([128, KC, HG], F32R)
        nc.sync.dma_start(out=wv_sb, in_=wv_d.bitcast(F32R).rearrange(
            "(kc p) f -> p kc f", p=128))

        xnT = xnpool.tile([128, KC, N], F32R)

        ropeq = persist.tile([128, 2, N], F32R)
        ropek = persist.tile([128, 2, N], F32R)
        vaug = persist.tile([128, NT, HPC, DH + 1], F32R)
        ctxn = persist.tile([128, 2, N], F32R)
        # ones column of v_aug (memset cannot write fp32r; DMA from DRAM)
        nc.sync.dma_start(
            out=vaug[:, :, :, DH:DH + 1],
            in_=onez_d.bitcast(F32R)[:, 0:NT * HPC].rearrange(
                "p (j h o) -> p j h o", j=NT, h=HPC))

        # ================= Phase 1: LayerNorm + transpose =================
        with tc.tile_pool(name="ph1", bufs=3) as ph1, \
             tc.tile_pool(name="ph1s", bufs=4) as ph1s, \
             tc.tile_pool(name="tp_ps", bufs=4, space="PSUM") as tp_ps:
            for it in range(NT):
                x_t = ph1.tile([128, DIM], F32)
                nc.sync.dma_start(out=x_t, in_=x_d[it * 128:(it + 1) * 128, :])
                stats = ph1s.tile([128, 2, 6], F32)
                nc.vector.bn_stats(out=stats[:, 0, :], in_=x_t[:, 0:512])
                nc.vector.bn_stats(out=stats[:, 1, :], in_=x_t[:, 512:1024])
                mv = ph1s.tile([128, 2], F32)
                nc.vector.bn_aggr(out=mv, in_=stats)
                rstd = ph1s.tile([128, 1], F32)
                nc.scalar.activation(out=rstd, in_=mv[:, 1:2], func=AF.Sqrt,
                                     bias=eps_sb)
                nc.vector.reciprocal(out=rstd, in_=rstd)
                xn_t = ph1.tile([128, DIM], F32R)
                nc.vector.tensor_scalar(out=xn_t, in0=x_t,
                                        scalar1=mv[:, 0:1], scalar2=rstd,
                                        op0=ALU.subtract, op1=ALU.mult)
                for half in range(2):
                    tp = tp_ps.tile([128, 512], F32R)
                    for b in range(4):
                        kc = half * 4 + b
                        nc.tensor.transpose(tp[:, b * 128:(b + 1) * 128],
                                            xn_t[:, kc * 128:(kc + 1) * 128],
                                            ident_sb)
                    # copy psum -> xnT[:, kc, it*128 : ...] for the 4 chunks
                    dst = xnT[:, half * 4:(half + 1) * 4,
                              it * 128:(it + 1) * 128]
                    src = tp.rearrange("p (b f) -> p b f", b=4)
                    if it % 4 == 3:
                        nc.scalar.copy(dst, src)
                    else:
                        nc.vector.tensor_copy(dst, src)

        # ================= Phase 2: QKV projections + RoPE + V ============
        with tc.tile_pool(name="qkv_ps", bufs=1, space="PSUM") as qkv_ps, \
             tc.tile_pool(name="vt_ps", bufs=2, space="PSUM") as vt_ps, \
             tc.tile_pool(name="ph2", bufs=2) as ph2:
            for kind, w_sb, of in (("q", wq_sb, 0), ("q", wq_sb, 1),
                                   ("k", wk_sb, 0), ("k", wk_sb, 1),
                                   ("v", wv_sb, 0), ("v", wv_sb, 1)):
                psums = []
                for c in range(CH):
                    ps = qkv_ps.tile([128, 512], F32, name=f"qkvps{c}",
                                     tag=f"qkvps{c}")
                    psums.append(ps)
                for kc in range(KC):
                    for c in range(CH):
                        nc.tensor.matmul(
                            psums[c],
                            w_sb[:, kc, of * 128:(of + 1) * 128],
                            xnT[:, kc, c * 512:(c + 1) * 512],
                            start=(kc == 0), stop=(kc == KC - 1))
                for c in range(CH):
                    cs = slice(c * 512, (c + 1) * 512)
                    if kind in ("q", "k"):
                        dest = ropeq if kind == "q" else ropek
                        qtmp = ph2.tile([128, 512], F32, tag="qtmp")
                        nc.vector.tensor_copy(qtmp, psums[c])
                        qshuf = ph2.tile([128, 512], F32, tag="qshuf")
                        for g in range(4):
                            nc.sync.dma_start(
                                out=qshuf[g * 32:(g + 1) * 32, :],
                                in_=qtmp[(g ^ 1) * 32:((g ^ 1) + 1) * 32, :])
                        t1 = ph2.tile([128, 512], F32, tag="t1")
                        nc.vector.tensor_mul(t1, qshuf, sin_sb[:, cs])
                        t2 = ph2.tile([128, 512], F32, tag="t2")
                        nc.vector.tensor_mul(t2, psums[c], cos_sb[:, cs])
                        nc.vector.tensor_add(dest[:, of, cs], t1, t2)
                    else:
                        vtmp = ph2.tile([128, 512], F32R, tag="vtmp")
                        nc.scalar.copy(vtmp, psums[c])
                        for b in range(4):
                            j = c * 4 + b
                            vt = vt_ps.tile([128, 128], F32R, tag="vt")
                            nc.tensor.transpose(
                                vt, vtmp[:, b * 128:(b + 1) * 128], ident_sb)
                            nc.vector.tensor_copy(
                                vaug[:, j, of * 2:of * 2 + 2, 0:DH],
                                vt.rearrange("p (h d) -> p h d", h=2))

        # ================= Phase 3: causal attention ======================
        with tc.tile_pool(name="s_ps", bufs=4, space="PSUM") as s_ps, \
             tc.tile_pool(name="ctxA_ps", bufs=1, space="PSUM") as ctxA_ps, \
             tc.tile_pool(name="ctxB_ps", bufs=1, space="PSUM") as ctxB_ps, \
             tc.tile_pool(name="ph3", bufs=6) as ph3, \
             tc.tile_pool(name="ph3s", bufs=4) as ph3s, \
             tc.tile_pool(name="dsc", bufs=4, space="DRAM") as dsc:
            for p in range(2):
                for c in range(CH):
                    cs = slice(c * 512, (c + 1) * 512)
                    nj = 4 * (c + 1)
                    ctxps = []
                    for hi, cpool in ((0, ctxA_ps), (1, ctxB_ps)):
                        ctxps.append(cpool.tile([DH + 1, 512], F32,
                                                name=f"ctxp{hi}",
                                                tag=f"ctxp{hi}"))
                    for j in range(nj):
                        for hi in range(2):
                            h = 2 * p + hi
                            off = hi * DH
                            sp = s_ps.tile([128, 512], F32, tag="sp")
                            nc.tensor.matmul(
                                sp,
                                ropek[off:off + DH, p, j * 128:(j + 1) * 128],
                                ropeq[off:off + DH, p, cs],
                                start=True, stop=True,
                                tile_position=(off, 0))
                            a_t = ph3.tile([128, 512], F32R, tag="a_t")
                            dj = j - 4 * c
                            if dj >= 0:
                                if dj > 0:
                                    nc.sync.dma_start(
                                        out=a_t[:, 0:dj * 128],
                                        in_=zer_sb[:, 0:dj * 128])
                                nc.scalar.activation(
                                    out=a_t[:, dj * 128:512],
                                    in_=sp[:, dj * 128:512],
                                    func=AF.Exp, scale=float(SCALE))
                                nc.vector.tensor_mul(
                                    a_t[:, dj * 128:(dj + 1) * 128],
                                    a_t[:, dj * 128:(dj + 1) * 128],
                                    tri_sb)
                            else:
                                nc.scalar.activation(out=a_t, in_=sp,
                                                     func=AF.Exp,
                                                     scale=float(SCALE))
                            nc.tensor.matmul(
                                ctxps[hi], vaug[:, j, h, :], a_t,
                                start=(j == 0), stop=(j == nj - 1))
                    for hi in range(2):
                        h = 2 * p + hi
                        off = hi * DH
                        recip = ph3s.tile([1, 512], F32, tag="recip")
                        nc.vector.reciprocal(recip, ctxps[hi][DH:DH + 1, :])
                        dtmp = dsc.tile([1, 512], F32, tag="dtmp")
                        nc.sync.dma_start(out=dtmp, in_=recip)
                        rb = ph3s.tile([DH, 512], F32, tag="rb")
                        bcast = bass.AP(tensor=dtmp.tensor, offset=dtmp.offset,
                                        ap=[[0, DH]] + list(dtmp.ap[1:]))
                        nc.sync.dma_start(out=rb, in_=bcast)
                        nc.vector.tensor_mul(ctxn[off:off + DH, p, cs],
                                             ctxps[hi][0:DH, :], rb)

        # ================= Phase 4: output projection =====================
        with tc.tile_pool(name="wo_ps", bufs=4, space="PSUM") as wo_ps, \
             tc.tile_pool(name="ph4", bufs=4) as ph4:
            for it in range(NT):
                for nh in range(2):
                    op = wo_ps.tile([128, 512], F32, tag="op")
                    for pc in range(2):
                        nc.tensor.matmul(
                            op, ctxn[:, pc, it * 128:(it + 1) * 128],
                            wo_sb[:, pc, nh * 512:(nh + 1) * 512],
                            start=(pc == 0), stop=(pc == 1))
                    ocp = ph4.tile([128, 512], F32, tag="ocp")
                    if (it * 2 + nh) % 2 == 0:
                        nc.vector.tensor_copy(ocp, op)
                    else:
                        nc.scalar.copy(ocp, op)
                    nc.sync.dma_start(
                        out=out_d[it * 128:(it + 1) * 128,
                                  nh * 512:(nh + 1) * 512],
                        in_=ocp)


def make_in_maps(x, gamma, beta, Wq, Wkv, Wo):
    x = np.asarray(x, dtype=np.float32)
    gamma = np.asarray(gamma, dtype=np.float32)
    beta = np.asarray(beta, dtype=np.float32)
    Wq = np.asarray(Wq, dtype=np.float32)
    Wkv = np.asarray(Wkv, dtype=np.float32)
    Wo = np.asarray(Wo, dtype=np.float32)
    if np.any(beta != 0.0):
        raise NotImplementedError("nonzero beta not supported by this kernel")
    wq_f = gamma[:, None] * Wq                       # fold gamma into weights
    wk_f = gamma[:, None] * Wkv[:, :DIM]
    wv_f = gamma[:, None] * Wkv[:, DIM:]
    cos128, sin128 = _rope_tables()
    tri = np.triu(np.ones((128, 128), dtype=np.float32))  # valid: k <= q
    ident = np.eye(128, dtype=np.float32)
    in_maps = []
    for core in range(N_CORES):
        b, hg = divmod(core, 4)
        sl = slice(hg * HG, (hg + 1) * HG)
        in_maps.append({
            "x": np.ascontiguousarray(x[b]),
            "wq": np.ascontiguousarray(wq_f[:, sl]),
            "wk": np.ascontiguousarray(wk_f[:, sl]),
            "wv": np.ascontiguousarray(wv_f[:, sl]),
            "wo": np.ascontiguousarray(Wo[sl, :]),
            "cos": cos128,
            "sin": sin128,
            "tri": tri,
            "ident": ident,
            "onez": np.ones((128, 384), dtype=np.float32),
        })
    return in_maps


def kernel(x, gamma, beta, Wq, Wkv, Wo, _trace=False):
    in_maps = make_in_maps(x, gamma, beta, Wq, Wkv, Wo)
    if "nc" not in _CACHE:
        _CACHE["nc"] = build_nc()
    nc = _CACHE["nc"]
    res = bass_utils.run_bass_kernel_spmd(
        nc, in_maps, core_ids=list(range(N_CORES)), trace=_trace)
    out = np.zeros((2, N, DIM), dtype=np.float64)
    for core in range(N_CORES):
        b = core // 4
        out[b] += res.results[core]["out"].astype(np.float64)
    _CACHE["last_results"] = res
    return out.astype(np.float32)
